# revision 5
# baseline (speedup 1.0000x reference)
"""Trainium2 Bass kernel for nn_ExemplarSoftmaxLoss (data-parallel over 8 cores).

Strategy (v2):
  - Shard batch dim B (and the 3 B-row blocks of `outputs`) across 8 cores.
  - Softmax part: per-row sum(exp(x)) via ScalarE Exp with row-accumulate;
    Exp's `out` (normally thrown away) is written as FP16 to SBUF, and the
    label logit is recovered from it: a fp16 (iota==label)*exp_x mask-STT on
    VectorE runs at 2x rate, and ln(exp(x_label)) = x_label (ln shares the
    exp ACT table set, so the recovery is one [128,96] ACT at the tail).
  - Distance part: exemplar rows gathered per 128-row block with [128,1]
    offset indirect DMAs; a custom DVE op computes a running prefix of
    (a-b+eps)^2 over each [128,4,512] pair in ONE VectorE pass; ScalarE
    extracts the prefix at the four 512-boundaries and the tail takes
    adjacent differences to get the four row-block square distances.
  - Host packs `outputs` as [16,128,3,1000] and a/p/n as [4,128,3,4,512] so
    every big load is a single contiguous DMA.
  - Host: float64 reduction of the 8x[128,4] partials -> 4 scalar losses.
"""

import os
import sys

import numpy as np

for _p in ("/opt/trn_rl_repo",):
    if _p not in sys.path and os.path.isdir(_p):
        sys.path.insert(0, _p)

import concourse.bass as bass
import concourse.tile as tile
from concourse import bacc, mybir
from concourse._compat import with_exitstack
from concourse.bass_utils import run_bass_kernel_spmd

# If BASS_TRACE is set in the environment, run_bass_kernel_spmd imports
# antenv.axon_hooks, which this image lacks -- stub it so we degrade to
# an untraced run instead of crashing.
try:
    import antenv.axon_hooks  # noqa: F401
except ImportError:
    import types as _types

    _m = _types.ModuleType("antenv.axon_hooks")
    _m.get_axon_ntff_profile_hook = lambda: None
    _m.set_axon_ntff_profile_hook = lambda h: None
    sys.modules["antenv.axon_hooks"] = _m

# Problem constants (hardcoded per the harness contract).
B, D, C = 16384, 512, 1000
NCORES = 8
BS = B // NCORES  # 2048 batch rows per core
RS = 3 * BS  # 6144 softmax rows per core
P = 128
NB = BS // P  # 16 row-blocks
NR = 3 * NB  # 48 (row-block, third) pairs
NG = 4  # groups of 4 row-blocks in the distance phase
EPS = 1e-6
MARGIN2 = 0.2
LAMBDA = 1.0

f32 = mybir.dt.float32
f16 = mybir.dt.float16
i32 = mybir.dt.int32
Alu = mybir.AluOpType
Act = mybir.ActivationFunctionType
AX = mybir.AxisListType

LAST_RESULTS = None  # BassKernelResults of the most recent run (for test.py)


# ---- custom DVE op: running prefix of (a - b + eps)^2 ----------------------
def _register_sqdiff_prefix():
    from concourse import dve_ops as dvo
    from concourse.dve_spec import AluOp, C0, Spec, Src0, Src1, lower, scan, sq
    from concourse.dve_uop import DveOpSpec

    name = "SQDIFF_PREFIX_ANT"
    for op in dvo.OPS:
        if op.name == name:
            return op

    def _ref(in0, in1, c0, c1, c2):
        a = np.asarray(in0, np.float32).reshape(in0.shape[0], -1)
        b = np.asarray(in1, np.float32).reshape(in0.shape[0], -1)
        c0v = c0 if isinstance(c0, float) else np.asarray(c0, np.float32)
        d = a - b + c0v
        return np.cumsum(d * d, axis=1)

    spec = Spec(body=scan(AluOp.ADD, sq(Src0 - Src1 + C0)), reference=_ref)
    row = max(dvo._SUB_OPCODE_FOR_NAME.values()) + 1
    assert row < 0x20
    dvo._SUB_OPCODE_FOR_NAME[name] = row
    uops = lower(spec, ver="v3")
    sha = DveOpSpec(name=name, opcode=row, uops=uops, rd1_en=True).sha("v3")
    op = dvo.DveOp(name, spec, subdim=False, uops_sha={"v3": sha})
    dvo.OPS.append(op)
    dvo.CUSTOM_DVE_SPECS[name] = spec
    return op


SQDIFF_OP = _register_sqdiff_prefix()


@with_exitstack
def _emit(ctx, tc, outs, ins):
    nc = tc.nc
    xo = ins["xout"]  # [NB, P, 3, C] f32  outputs, host-packed tiles
    apn = ins["apn"]  # [NG, P, 3, 4, D] f32  anchor/positive/negative packed
    ex = ins["exem"]  # [C, D] f32  exemplar table
    la = ins["lab_a"]  # [P, NB] i32  labels_anchor, row blk*128+p at [p, blk]
    ln_ = ins["lab_n"]  # [P, NB] i32  labels_neg
    lf = ins["lab_f"]  # [P, NR] f32  concat labels as f32, col rb = t*16 + blk
    pd = outs["partials"]  # [P, 4] f32

    sing = ctx.enter_context(tc.tile_pool(name="sing", bufs=1))
    xpool = ctx.enter_context(tc.tile_pool(name="xp", bufs=3))
    ejp = ctx.enter_context(tc.tile_pool(name="ejp", bufs=5))
    mop = ctx.enter_context(tc.tile_pool(name="mop", bufs=3))
    apnp = ctx.enter_context(tc.tile_pool(name="apnp", bufs=2))
    expool = ctx.enter_context(tc.tile_pool(name="expool", bufs=2))
    prefp = ctx.enter_context(tc.tile_pool(name="prefp", bufs=3))

    sl = sing.tile([P, 2 * NR], f32)  # cols 0..47 sum(exp), 48..95 exp(x_lbl)
    # prefix extracts, pair-major; col 0 of each group is a zero so the tail
    # can take adjacent differences with one subtract
    d2p = sing.tile([P, 6, NG, 5], f32)
    la_t = sing.tile([P, NB], i32)
    ln_t = sing.tile([P, NB], i32)
    lf_t = sing.tile([P, NR], f32)
    iota_h = sing.tile([P, C], f16)
    nc.gpsimd.memset(d2p[:], 0.0)

    # small loads via SWDGE so the Sync HWDGE queue leads with the x-tile stream
    nc.gpsimd.dma_start(out=la_t[:], in_=la[:])
    nc.gpsimd.dma_start(out=ln_t[:], in_=ln_[:])
    nc.gpsimd.dma_start(out=lf_t[:], in_=lf[:])
    nc.gpsimd.iota(
        iota_h[:],
        pattern=[[1, C]],
        base=0,
        channel_multiplier=0,
        allow_small_or_imprecise_dtypes=True,
    )

    def emit_gathers(g, exa, exn, b2s=range(4), exa_first=False):
        pairs_ = (
            [(exa, la_t, b2) for b2 in b2s] + [(exn, ln_t, b2) for b2 in b2s]
            if exa_first
            else [t for b2 in b2s for t in ((exa, la_t, b2), (exn, ln_t, b2))]
        )
        for dst, lab_t, b2 in pairs_:
            blk = 4 * g + b2
            nc.gpsimd.indirect_dma_start(
                out=dst[:, b2, :],
                out_offset=None,
                in_=ex[:],
                in_offset=bass.IndirectOffsetOnAxis(
                    ap=lab_t[:, blk : blk + 1], axis=0
                ),
            )

    def emit_apn_load(g):
        t = apnp.tile([P, 3, 4, D], f32, tag="apn", name=f"apn{g}")
        nc.sync.dma_start(out=t[:], in_=apn[g])
        return t

    # software-pipeline exemplar gathers + apn loads one group ahead
    ex_tiles = {
        0: (
            expool.tile([P, 4, D], f32, tag="exa", name="exa0"),
            expool.tile([P, 4, D], f32, tag="exn", name="exn0"),
        )
    }
    emit_gathers(0, *ex_tiles[0], exa_first=True)
    apn_tiles = {}

    def emit_xtile(i):
        xt = xpool.tile([P, 3, C], f32, tag="xt", name=f"xt{i}")
        nc.sync.dma_start(out=xt[:], in_=xo[i])
        for t in range(3):
            col = t * NB + i
            ej = ejp.tile([P, C], f16, tag="ej")
            nc.scalar.activation(
                out=ej[:],
                in_=xt[:, t, :],
                func=Act.Exp,
                accum_out=sl[:, col : col + 1],
            )
            mo = mop.tile([P, C], f16, tag="mo")
            nc.vector.scalar_tensor_tensor(
                out=mo[:],
                in0=iota_h[:],
                scalar=lf_t[:, col : col + 1],
                in1=ej[:],
                op0=Alu.is_equal,
                op1=Alu.mult,
                accum_out=sl[:, NR + col : NR + col + 1],
            )

    def emit_pair(g, apn_t, exa, exn, ci):
        a = apn_t[:, 0]
        pairs = (
            (a, exa[:]),  # d_ref1
            (apn_t[:, 2], exa[:]),  # d_neg1
            (a, exn[:]),  # d_ref2
            (apn_t[:, 2], exn[:]),  # d_neg2
            (a, apn_t[:, 1]),  # tp
            (a, apn_t[:, 2]),  # tn
        )
        xs, ys = pairs[ci]
        pref = prefp.tile([P, 4, D], f32, tag="pref")
        nc.vector._custom_dve(SQDIFF_OP, out=pref[:], in0=xs, in1=ys, s0=EPS)
        # prefix at each 512-boundary -> d2p; tail takes adjacent diffs
        nc.scalar.copy(
            out=d2p[:, ci, g, 1:5],
            in_=pref[:, :, D - 1 : D].rearrange("p a b -> p (a b)"),
        )

    # schedule: group g's 6 distance pairs run at absolute steps
    # 4g+2, 4g+3, 4g+4 (2 pairs each) so its apn load + gathers have landed
    pair_sched = {}
    for g in range(NG):
        for j in range(3):
            pair_sched.setdefault(4 * g + 2 + j, []).extend(
                [(g, 2 * j), (g, 2 * j + 1)]
            )
    for i in range(NB):
        g, pi = divmod(i, 4)
        emit_xtile(i)
        if i == 0:
            apn_tiles[0] = emit_apn_load(0)
        if pi == 1 and g + 1 < NG:
            ex_tiles[g + 1] = (
                expool.tile([P, 4, D], f32, tag="exa", name=f"exa{g + 1}"),
                expool.tile([P, 4, D], f32, tag="exn", name=f"exn{g + 1}"),
            )
        if pi == 2 and g + 1 < NG:
            apn_tiles[g + 1] = emit_apn_load(g + 1)
        if pi >= 2 and g + 1 < NG:
            # spread next group's gathers: 4 indirect DMAs per step
            emit_gathers(g + 1, *ex_tiles[g + 1], b2s=[2 * (pi - 2), 2 * (pi - 2) + 1])
        for pg, ci in pair_sched.get(i, []):
            emit_pair(pg, apn_tiles[pg], *ex_tiles[pg], ci)
    for pg, ci in pair_sched.get(NB, []):
        emit_pair(pg, apn_tiles[pg], *ex_tiles[pg], ci)

    # ---- tail ----
    part = sing.tile([P, 4], f32)
    logs = sing.tile([P, 2 * NR], f32)
    # ln first (shares the exp ACT table set), then ONE switch to sqrt
    nc.scalar.activation(out=logs[:], in_=sl[:], func=Act.Ln)
    nc.vector.reduce_sum(out=part[:, 0:1], in_=logs[:, 0:NR], axis=AX.X)
    nc.vector.reduce_sum(out=part[:, 1:2], in_=logs[:, NR : 2 * NR], axis=AX.X)

    # adjacent diffs of the per-group prefixes -> block square-distances
    d2f = sing.tile([P, 6, NG, 4], f32)
    nc.vector.tensor_tensor(
        out=d2f[:], in0=d2p[:, :, :, 1:5], in1=d2p[:, :, :, 0:4],
        op=Alu.subtract,
    )
    dd = sing.tile([P, 6, NB], f32)
    nc.scalar.activation(
        out=dd[:].rearrange("p c n -> p (c n)"),
        in_=d2f[:].rearrange("p c g b -> p (c g b)"),
        func=Act.Sqrt,
    )

    x1 = sing.tile([P, NB], f32)
    m1 = sing.tile([P, NB], f32)
    c1 = sing.tile([P, NB], f32)
    x2 = sing.tile([P, NB], f32)
    c2 = sing.tile([P, NB], f32)
    x3 = sing.tile([P, NB], f32)
    t3 = sing.tile([P, NB], f32)
    ca = sing.tile([P, 1], f32)
    cb = sing.tile([P, 1], f32)

    # c1 = (dr1 - dn1 > 0) ? (dr1 - dn1 + MARGIN2) : 0
    nc.vector.tensor_tensor(out=x1[:], in0=dd[:, 0, :], in1=dd[:, 1, :], op=Alu.subtract)
    nc.vector.tensor_scalar(
        out=m1[:], in0=x1[:], scalar1=0.0, scalar2=None, op0=Alu.is_gt
    )
    nc.vector.scalar_tensor_tensor(
        out=c1[:], in0=x1[:], scalar=MARGIN2, in1=m1[:],
        op0=Alu.add, op1=Alu.mult, accum_out=ca[:],
    )
    # c2 = relu(dn2 - dr2)
    nc.vector.tensor_tensor(out=x2[:], in0=dd[:, 3, :], in1=dd[:, 2, :], op=Alu.subtract)
    nc.vector.tensor_scalar(
        out=c2[:], in0=x2[:], scalar1=0.0, scalar2=None,
        op0=Alu.max, op1=Alu.add, accum_out=cb[:],
    )
    # t = relu(tp - tn)
    nc.vector.tensor_tensor(out=x3[:], in0=dd[:, 4, :], in1=dd[:, 5, :], op=Alu.subtract)
    nc.vector.tensor_scalar(
        out=t3[:], in0=x3[:], scalar1=0.0, scalar2=None,
        op0=Alu.max, op1=Alu.add, accum_out=part[:, 3:4],
    )
    nc.vector.tensor_tensor(out=part[:, 2:3], in0=ca[:], in1=cb[:], op=Alu.add)
    nc.sync.dma_start(out=pd[:], in_=part[:])


_COMPILED = None


def _build():
    global _COMPILED
    if _COMPILED is not None:
        return _COMPILED
    nc = bacc.Bacc(
        "TRN2",
        target_bir_lowering=False,
        debug=False,
        enable_asserts=False,
        num_devices=NCORES,
    )
    ins = {
        "xout": nc.dram_tensor("xout", [NB, P, 3, C], f32, kind="ExternalInput").ap(),
        "apn": nc.dram_tensor("apn", [NG, P, 3, 4, D], f32, kind="ExternalInput").ap(),
        "exem": nc.dram_tensor("exem", [C, D], f32, kind="ExternalInput").ap(),
        "lab_a": nc.dram_tensor("lab_a", [P, NB], i32, kind="ExternalInput").ap(),
        "lab_n": nc.dram_tensor("lab_n", [P, NB], i32, kind="ExternalInput").ap(),
        "lab_f": nc.dram_tensor("lab_f", [P, NR], f32, kind="ExternalInput").ap(),
    }
    outs = {
        "partials": nc.dram_tensor("partials", [P, 4], f32, kind="ExternalOutput").ap()
    }
    with tile.TileContext(nc) as tc:
        _emit(tc, outs, ins)
    nc.compile()
    _COMPILED = nc
    return nc


def _in_maps(anchor, positive, negative, outputs, labels_anchor, labels_neg, exemplars):
    anchor = np.asarray(anchor, np.float32)
    positive = np.asarray(positive, np.float32)
    negative = np.asarray(negative, np.float32)
    outputs = np.asarray(outputs, np.float32)
    exemplars = np.ascontiguousarray(np.asarray(exemplars, np.float32))
    la_all = np.asarray(labels_anchor).astype(np.int64)
    ln_all = np.asarray(labels_neg).astype(np.int64)

    maps = []
    for k in range(NCORES):
        sl_ = slice(k * BS, (k + 1) * BS)
        la, ln = la_all[sl_], ln_all[sl_]
        # [3, BS, C] -> [NB, P, 3, C]
        xo3 = np.stack(
            [
                outputs[k * BS : (k + 1) * BS],
                outputs[B + k * BS : B + (k + 1) * BS],
                outputs[2 * B + k * BS : 2 * B + (k + 1) * BS],
            ],
            axis=0,
        )
        xo = np.ascontiguousarray(
            xo3.reshape(3, NB, P, C).transpose(1, 2, 0, 3)
        )
        # a/p/n [BS, D] -> [NG, P, 3, 4, D]
        apn3 = np.stack([anchor[sl_], positive[sl_], negative[sl_]], axis=0)
        apn = np.ascontiguousarray(
            apn3.reshape(3, NG, 4, P, D).transpose(1, 3, 0, 2, 4)
        )
        labels_cat = np.concatenate([la, la, ln])
        maps.append(
            {
                "xout": xo,
                "apn": apn,
                "exem": exemplars,
                "lab_a": np.ascontiguousarray(la.reshape(NB, P).T.astype(np.int32)),
                "lab_n": np.ascontiguousarray(ln.reshape(NB, P).T.astype(np.int32)),
                "lab_f": np.ascontiguousarray(
                    labels_cat.reshape(NR, P).T.astype(np.float32)
                ),
            }
        )
    return maps


def _combine(results):
    S = np.zeros(4, dtype=np.float64)
    for r in results:
        S += r["partials"].astype(np.float64).sum(axis=0)
    loss_softmax = (S[0] - S[1]) / (3 * B)
    loss_center = S[2]
    loss_triplet = S[3]
    loss_total = loss_softmax + 0.01 * loss_center + LAMBDA * loss_triplet
    return (
        np.float32(loss_total),
        np.float32(loss_triplet),
        np.float32(loss_softmax),
        np.float32(loss_center),
    )


def kernel(anchor, positive, negative, outputs, labels_anchor, labels_neg, exemplars):
    global LAST_RESULTS
    nc = _build()
    maps = _in_maps(
        anchor, positive, negative, outputs, labels_anchor, labels_neg, exemplars
    )
    res = run_bass_kernel_spmd(nc, maps, core_ids=list(range(NCORES)))
    LAST_RESULTS = res
    return _combine(res.results)
